# revision 5
# baseline (speedup 1.0000x reference)
"""Trainium2 Bass kernel for nn_BasicNet4 (Emformer encoder, sparse attention).

Strategy (v2):
  - Data-parallel over batch B=8 across 8 NeuronCores (weights replicated).
  - Tokens reordered host-side into segment-interleaved order:
    seg i -> [rc_i, u_{4i}..u_{4i+3}] (5 tokens x 256 segs = 1280), so
    attention is block-diagonal with 5x5 blocks.
  - Attention via SEGMENT-ALIGNED windows of 25 segs = 125 tokens (10 full
    windows + one 30-token remainder). No halos/edges; the in-window mask is
    one rank-26 matmul accumulated into each score chunk. Since the pa
    partition index is the within-window key slot, the softmax denominator is
    a single 125-partition column-sum over the full token width.
  - Activations transposed in SBUF [d on partitions (4x128), tokens free].
  - LN affines folded into weights host-side. Layer-0 ln_in computed on the
    HOST. When the LN affines are trivial (they are for the reference
    inputs; runtime-checked), ln_out output is already normalized, so
    ln_in for layers 1..3 vanishes on device (z = cat).
  - ff_ln folded into the FFN-W1 matmul: h1 = relu((W1@rc - s_w1*mu)*rstd+b1)
    using a rank-1 correction, so zf is never materialized.
  - All reciprocals via DVE reciprocal_approx_fast (fp32), not the slow
    iterative InstReciprocal.
  - bf16 matmul operands / residual stream, fp32 PSUM accumulation.
"""

import sys

sys.path.insert(0, "/opt/trn_rl_repo")

import numpy as np
import ml_dtypes

import concourse.bass as bass
import concourse.mybir as mybir
import concourse.tile as tile
from concourse import bass_utils, bacc

bf16 = ml_dtypes.bfloat16
dt = mybir.dt
AF = mybir.ActivationFunctionType
ALU = mybir.AluOpType

# Model config (hardcoded from the problem spec)
D, H, FFN, L = 512, 4, 128, 4
SEG, RC = 4, 1
B, T = 8, 1025
U = T - RC            # 1024
NSEG = U // SEG       # 256
TT = NSEG * (SEG + RC)  # 1280 interleaved tokens
DT = D // 128         # 4 d tiles
DH = D // H           # 128 (= one partition tile per head)
NCORES = 8
CHUNKS = [(0, 512), (512, 512), (1024, 256)]  # psum-bank-sized fp32 chunks

# Attention windows: 25 segments = 125 tokens each; last window = 6 segs = 30.
WW = 125
NWIN = 11
WIN = [(WW * w, WW if w < NWIN - 1 else TT - WW * (NWIN - 1)) for w in range(NWIN)]
# score/AV psum groups: windows per group, and their column spans
GROUPS = [(0, 4), (4, 4), (8, 3)]   # (first window, n windows)
GSPAN = [(WIN[w0][0], sum(WIN[w0 + i][1] for i in range(nw))) for (w0, nw) in GROUPS]

CBF = np.float32(bf16(np.float32(30000.0)))  # mask constant (exp(-30000) == 0)

_COMPILED = {}


def _tok_index():
    # interleaved token t -> original frame index in x[:, :T]
    t = np.arange(TT)
    seg = t // 5
    pos = t % 5
    off = np.array([4, 0, 1, 2, 3])[pos]
    return 4 * seg + off  # in [0, 1024]


def _mask_consts():
    """Window mask factors: psum += Lm.T @ Rm gives, for k-slot t and global
    query column j (window w = j//125, pos = j-125w):
        -C + C*[t//5 == pos//5]
    so allowed (same in-window segment) pairs get 0 and the rest -C.
    k-slots beyond the remainder window's 30 valid tokens get plain -C."""
    NS = WW // 5  # 25 segments per full window
    Lm = np.zeros((1 + NS, WW), np.float32)
    Rm = np.zeros((1 + NS, TT), np.float32)
    Lm[0] = 1.0
    for s in range(NS):
        Lm[1 + s, 5 * s:5 * s + 5] = 1.0
    j = np.arange(TT)
    w = np.minimum(j // WW, NWIN - 1)
    pos = j - WW * w
    Rm[0] = -CBF
    for s in range(NS):
        Rm[1 + s] = CBF * (pos // 5 == s)
    return Lm.astype(bf16), Rm.astype(bf16)


def _ln_host(x):
    mu = x.mean(-1, keepdims=True)
    var = ((x - mu) ** 2).mean(-1, keepdims=True)
    return (x - mu) / np.sqrt(var + 1e-5)


def _host_prep(ins):
    """Fold LN affines into weights, transpose, cast. Returns shared input map
    and whether the degenerate-affine fast path applies."""
    f32 = np.float32
    m = {}
    scale = np.float32(DH) ** -0.5
    degen = all(
        np.all(f32(ins[g]) == 1.0) and np.all(f32(ins[b]) == 0.0)
        for g, b in [("ln_in_g", "ln_in_b"), ("ln_out_g", "ln_out_b")]
    )
    for l in range(L):
        g_i, b_i = f32(ins["ln_in_g"][l]), f32(ins["ln_in_b"][l])
        g_f, b_f = f32(ins["ff_ln_g"][l]), f32(ins["ff_ln_b"][l])
        Wq = f32(ins["Wq"][l]);  bq = f32(ins["bq"][l])
        Wk = f32(ins["Wkv"][l][:D]);  bk = f32(ins["bkv"][l][:D])
        Wv = f32(ins["Wkv"][l][D:]);  bv = f32(ins["bkv"][l][D:])
        Wo = f32(ins["Wo"][l]);  bo = f32(ins["bo"][l])
        W1 = f32(ins["W1"][l]);  b1 = f32(ins["b1"][l])
        W2 = f32(ins["W2"][l]);  b2 = f32(ins["b2"][l])
        Wq_ = scale * (Wq * g_i[None, :]); bq_ = scale * (bq + Wq @ b_i)
        Wk_ = Wk * g_i[None, :];           bk_ = bk + Wk @ b_i
        Wv_ = Wv * g_i[None, :];           bv_ = bv + Wv @ b_i
        W1_ = W1 * g_f[None, :];           b1_ = b1 + W1 @ b_f
        m[f"wq{l}"] = Wq_.T.copy().astype(bf16)   # [din, dout]
        m[f"wk{l}"] = Wk_.T.copy().astype(bf16)
        m[f"wv{l}"] = Wv_.T.copy().astype(bf16)
        m[f"wo{l}"] = Wo.T.copy().astype(bf16)
        m[f"w1{l}"] = W1_.T.copy().astype(bf16)   # [512, 128]
        m[f"w2{l}"] = W2.T.copy().astype(bf16)    # [128, 512]
        m[f"bq{l}"] = bq_.reshape(DT, 128).T.copy()       # [128, DT] f32
        m[f"bk{l}"] = bk_.reshape(DT, 128).T.copy()
        m[f"bv{l}"] = bv_.reshape(1, D).astype(bf16)      # [1, 512] row
        m[f"bo{l}"] = bo.reshape(DT, 128).T.copy()
        m[f"b1{l}"] = b1_.reshape(1, 128).T.copy()        # [128, 1]
        m[f"sw1{l}"] = (-W1_.sum(axis=1)).reshape(1, 128).T.copy()  # [128, 1]
        m[f"b2{l}"] = b2.reshape(DT, 128).T.copy()
        m[f"go{l}"] = f32(ins["ln_out_g"][l]).reshape(DT, 128).T.copy()
        m[f"bo2{l}"] = f32(ins["ln_out_b"][l]).reshape(DT, 128).T.copy()
    Lm, Rm = _mask_consts()
    m["lm"] = Lm                                     # [26, 125]
    m["rm"] = Rm                                     # [26, 1280]
    m["ones_c"] = np.full((128, 128), 1.0 / D, bf16)  # stats lhsT (bcast)
    m["ones_w"] = np.ones((WW, 128), bf16)            # denominator lhsT
    m["ones1"] = np.ones((1, WW), bf16)               # K=1 bcast lhsT (V bias)
    m["ident"] = np.eye(128, dtype=bf16)              # residual adds
    return m, degen


def _dram_inputs(nc, degen):
    a = {}
    def inp(name, shape, dtype):
        a[name] = nc.dram_tensor(name, list(shape), dtype, kind="ExternalInput").ap()
    inp("xT", (D, TT), dt.bfloat16)
    inp("zT", (D, TT), dt.bfloat16)  # host-side ln_in of layer 0
    for l in range(L):
        inp(f"wq{l}", (D, D), dt.bfloat16); inp(f"wk{l}", (D, D), dt.bfloat16)
        inp(f"wv{l}", (D, D), dt.bfloat16); inp(f"wo{l}", (D, D), dt.bfloat16)
        inp(f"w1{l}", (D, FFN), dt.bfloat16); inp(f"w2{l}", (FFN, D), dt.bfloat16)
        inp(f"bq{l}", (128, DT), dt.float32); inp(f"bk{l}", (128, DT), dt.float32)
        inp(f"bv{l}", (1, D), dt.bfloat16); inp(f"bo{l}", (128, DT), dt.float32)
        inp(f"b1{l}", (128, 1), dt.float32); inp(f"sw1{l}", (128, 1), dt.float32)
        inp(f"b2{l}", (128, DT), dt.float32)
        if not degen:
            inp(f"go{l}", (128, DT), dt.float32); inp(f"bo2{l}", (128, DT), dt.float32)
    inp("lm", (1 + WW // 5, WW), dt.bfloat16)
    inp("rm", (1 + WW // 5, TT), dt.bfloat16)
    inp("ones_c", (128, 128), dt.bfloat16)
    inp("ones_w", (WW, 128), dt.bfloat16)
    inp("ones1", (1, WW), dt.bfloat16)
    inp("ident", (128, 128), dt.bfloat16)
    out = nc.dram_tensor("out", [128, DT], dt.float32, kind="ExternalOutput").ap()
    return a, out


def _ln_stats(nc, pools, smalls, src, sq, eps_tile):
    """Given src [128, DT, TT] bf16 and its squares sq (same shape), return
    (mu_b bf16 [128, TT] broadcast, A fp32 [128, TT] broadcast = 1/std)."""
    acts, sbufs, psums = pools
    ones_c = smalls["ones_c"]
    mu_b = sbufs.tile([128, TT], dt.bfloat16, tag="mu")
    sqmu = sbufs.tile([128, TT], dt.float32, tag="sqmu")
    var = sbufs.tile([128, TT], dt.float32, tag="var")
    A = sbufs.tile([128, TT], dt.float32, tag="A")
    for (c0, cn) in CHUNKS:
        p_mu = psums.tile([128, cn], dt.float32, tag="pc")
        p_e2 = psums.tile([128, cn], dt.float32, tag="pc")
        for d in range(DT):
            nc.tensor.matmul(p_mu[:], ones_c[:], src[:, d, c0:c0 + cn],
                             start=(d == 0), stop=(d == DT - 1))
        for d in range(DT):
            nc.tensor.matmul(p_e2[:], ones_c[:], sq[:, d, c0:c0 + cn],
                             start=(d == 0), stop=(d == DT - 1))
        nc.vector.tensor_copy(mu_b[:, c0:c0 + cn], p_mu[:])
        nc.scalar.activation(sqmu[:, c0:c0 + cn], p_mu[:], AF.Square)
        nc.vector.tensor_tensor(var[:, c0:c0 + cn], p_e2[:], sqmu[:, c0:c0 + cn],
                                ALU.subtract)
        # std = sqrt(var + eps); A = 1/std
        nc.scalar.activation(sqmu[:, c0:c0 + cn], var[:, c0:c0 + cn], AF.Sqrt,
                             bias=eps_tile[:], scale=1.0)
        nc.vector.reciprocal_approx_fast(A[:, c0:c0 + cn], sqmu[:, c0:c0 + cn])
    return mu_b, A


def _squares(nc, acts, src, split=2):
    """sq = src*src elementwise, [128, DT, TT] bf16; split across engines."""
    sq = acts.tile([128, DT, TT], dt.bfloat16, tag="sq")
    for d in range(DT):
        if d < split:
            nc.gpsimd.tensor_tensor(sq[:, d], src[:, d], src[:, d], ALU.mult)
        else:
            nc.scalar.activation(sq[:, d], src[:, d], AF.Square)
    return sq


def _trace(nc, degen):
    a, out_dram = _dram_inputs(nc, degen)
    with tile.TileContext(nc) as tc:
        import contextlib
        ctx = contextlib.ExitStack()
        with ctx:
            consts = ctx.enter_context(tc.tile_pool(name="consts", bufs=1))
            wpool = ctx.enter_context(tc.tile_pool(name="w", bufs=2))
            acts = ctx.enter_context(tc.tile_pool(name="acts", bufs=1))
            sbufs = ctx.enter_context(tc.tile_pool(name="sbufs", bufs=2))
            # PSUM budget: 8 banks of 2KB. pc ring x4 + pv x1 + ps x3.
            psums = ctx.enter_context(tc.tile_pool(name="psums", bufs=4, space="PSUM"))
            pvp = ctx.enter_context(tc.tile_pool(name="pvp", bufs=1, space="PSUM"))
            psp = ctx.enter_context(tc.tile_pool(name="psp", bufs=3, space="PSUM"))

            # constants
            smalls = {}
            for name, shape, dd in [
                ("lm", [1 + WW // 5, WW], dt.bfloat16),
                ("rm", [1 + WW // 5, TT], dt.bfloat16),
                ("ones_c", [128, 128], dt.bfloat16),
                ("ones_w", [WW, 128], dt.bfloat16),
                ("ones1", [1, WW], dt.bfloat16),
                ("ident", [128, 128], dt.bfloat16),
            ]:
                t = consts.tile(shape, dd, tag=name)
                nc.sync.dma_start(t[:], a[name])
                smalls[name] = t
            eps_tile = consts.tile([128, 1], dt.float32)
            nc.vector.memset(eps_tile[:], 1e-5)

            # initial residual stream + host-normalized layer-0 LN input
            cat = acts.tile([128, DT, TT], dt.bfloat16, tag="cat0")
            nc.sync.dma_start(cat[:], a["xT"].rearrange("(dtile p) t -> p dtile t", p=128))
            z0 = acts.tile([128, DT, TT], dt.bfloat16, tag="z0")
            nc.sync.dma_start(z0[:], a["zT"].rearrange("(dtile p) t -> p dtile t", p=128))

            pools = (acts, sbufs, psums)

            for l in range(L):
                # --- load layer weights ---
                w = {}
                for nm, shape in [("wq", [128, DT, D]), ("wk", [128, DT, D]),
                                  ("wv", [128, DT, D]), ("wo", [128, DT, D]),
                                  ("w1", [128, DT, FFN]), ("w2", [128, D])]:
                    t = wpool.tile(shape, dt.bfloat16, tag=nm)
                    src = a[f"{nm}{l}"]
                    if nm == "w2":
                        nc.sync.dma_start(t[:], src)
                    else:
                        nc.sync.dma_start(t[:], src.rearrange("(dtile p) o -> p dtile o", p=128))
                    w[nm] = t
                bias = {}
                bnames = ["bq", "bk", "bo", "b1", "sw1", "b2"]
                if not degen:
                    bnames += ["go", "bo2"]
                for nm in bnames:
                    t = wpool.tile([128, DT] if nm not in ("b1", "sw1") else [128, 1],
                                   dt.float32, tag=nm)
                    nc.sync.dma_start(t[:], a[f"{nm}{l}"])
                    bias[nm] = t
                bv = wpool.tile([1, D], dt.bfloat16, tag="bv")
                nc.sync.dma_start(bv[:], a[f"bv{l}"])

                # --- ln_in -> z ---
                if l == 0:
                    z = z0
                elif degen:
                    z = cat           # ln_out output is already normalized
                else:
                    sqc = _squares(nc, acts, cat)
                    mu_b, A = _ln_stats(nc, pools, smalls, cat, sqc, eps_tile)
                    z = acts.tile([128, DT, TT], dt.bfloat16, tag="z")
                    for d in range(DT):
                        xc = sbufs.tile([128, TT], dt.bfloat16, tag="xc")
                        nc.vector.tensor_tensor(xc[:], cat[:, d], mu_b[:], ALU.subtract)
                        nc.vector.tensor_tensor(z[:, d], xc[:], A[:], ALU.mult)

                # --- Q, K projections (weights stationary -> transposed out) ---
                qk = {}
                for nm, bnm in [("wq", "bq"), ("wk", "bk")]:
                    dst = acts.tile([128, DT, TT], dt.bfloat16,
                                    tag="q" if nm == "wq" else "k")
                    for o in range(DT):
                        for (c0, cn) in CHUNKS:
                            p = psums.tile([128, cn], dt.float32, tag="pc")
                            for d in range(DT):
                                nc.tensor.matmul(
                                    p[:],
                                    w[nm][:, d, 128 * o:128 * o + 128],
                                    z[:, d, c0:c0 + cn],
                                    start=(d == 0), stop=(d == DT - 1))
                            nc.scalar.activation(dst[:, o, c0:c0 + cn], p[:],
                                                 AF.Identity,
                                                 bias=bias[bnm][:, o:o + 1], scale=1.0)
                    qk[nm] = dst
                q_t, k_t = qk["wq"], qk["wk"]

                # --- V projection (acts stationary -> natural [tok, d]) ---
                v_nat = acts.tile([WW, NWIN, D], dt.bfloat16, tag="v")
                for wi, (w0, wd) in enumerate(WIN):
                    p = pvp.tile([WW, D], dt.float32, tag="pv")
                    # bias first, writing all WW partitions (keeps remainder-
                    # window rows finite so 0*v stays 0 in the AV matmul)
                    nc.tensor.matmul(p[:], smalls["ones1"][:], bv[:],
                                     start=True, stop=False)
                    for d in range(DT):
                        nc.tensor.matmul(p[0:wd], z[:, d, w0:w0 + wd],
                                         w["wv"][:, d, :], start=False,
                                         stop=(d == DT - 1), skip_group_check=True)
                    nc.scalar.activation(v_nat[:, wi, :], p[:], AF.Identity)

                # --- attention per head ---
                attn = acts.tile([128, DT, TT], dt.bfloat16, tag="attn")
                for h in range(H):
                    pa = sbufs.tile([WW, TT], dt.bfloat16, tag="pa")
                    # scores + mask, grouped windows per psum bank
                    for gi, (gw0, gnw) in enumerate(GROUPS):
                        g0, gn = GSPAN[gi]
                        ps = psp.tile([WW, gn], dt.float32, tag="ps")
                        nc.tensor.matmul(ps[:], smalls["lm"][:],
                                         smalls["rm"][:, g0:g0 + gn],
                                         start=True, stop=False)
                        for k in range(gnw):
                            w0, wd = WIN[gw0 + k]
                            lo = w0 - g0
                            nc.tensor.matmul(ps[0:wd, lo:lo + wd],
                                             k_t[:, h, w0:w0 + wd],
                                             q_t[:, h, w0:w0 + wd],
                                             start=False, stop=(k == gnw - 1),
                                             skip_group_check=True)
                        nc.scalar.activation(pa[:, g0:g0 + gn], ps[:], AF.Exp)
                    # denominator: plain partition sum over the window axis
                    rec = sbufs.tile([128, TT], dt.float32, tag="rec")
                    for (c0, cn) in CHUNKS:
                        pd = psums.tile([128, cn], dt.float32, tag="pc")
                        nc.tensor.matmul(pd[:], smalls["ones_w"][:],
                                         pa[:, c0:c0 + cn], start=True, stop=True)
                        nc.vector.reciprocal_approx_fast(rec[:, c0:c0 + cn], pd[:])
                    # attn-value matmuls (grouped like scores)
                    for gi, (gw0, gnw) in enumerate(GROUPS):
                        g0, gn = GSPAN[gi]
                        pv = psp.tile([128, gn], dt.float32, tag="ps")
                        for k in range(gnw):
                            w0, wd = WIN[gw0 + k]
                            lo = w0 - g0
                            nc.tensor.matmul(pv[:, lo:lo + wd],
                                             v_nat[:, gw0 + k, 128 * h:128 * h + 128],
                                             pa[:, w0:w0 + wd],
                                             start=True, stop=True)
                        nc.vector.tensor_tensor(attn[:, h, g0:g0 + gn], pv[:],
                                                rec[:, g0:g0 + gn], ALU.mult)

                # --- Wo projection + residual ---
                rc = acts.tile([128, DT, TT], dt.bfloat16, tag="rc")
                for o in range(DT):
                    for (c0, cn) in CHUNKS:
                        p = psums.tile([128, cn], dt.float32, tag="pc")
                        for d in range(DT):
                            nc.tensor.matmul(p[:],
                                             w["wo"][:, d, 128 * o:128 * o + 128],
                                             attn[:, d, c0:c0 + cn],
                                             start=(d == 0), stop=False)
                        nc.tensor.matmul(p[:], smalls["ident"][:],
                                         cat[:, o, c0:c0 + cn], start=False, stop=True)
                        nc.scalar.activation(rc[:, o, c0:c0 + cn], p[:], AF.Identity,
                                             bias=bias["bo"][:, o:o + 1], scale=1.0)

                # --- ff_ln folded into W1: h1 = relu((W1@rc - sw1*mu)*A + b1) ---
                sqr = _squares(nc, acts, rc)
                mu_r, A_r = _ln_stats(nc, pools, smalls, rc, sqr, eps_tile)
                h1 = acts.tile([128, TT], dt.bfloat16, tag="h1")
                for (c0, cn) in CHUNKS:
                    p = psums.tile([128, cn], dt.float32, tag="pc")
                    for d in range(DT):
                        nc.tensor.matmul(p[:], w["w1"][:, d, :],
                                         rc[:, d, c0:c0 + cn],
                                         start=(d == 0), stop=(d == DT - 1))
                    u = sbufs.tile([128, cn], dt.float32, tag="u")
                    nc.vector.scalar_tensor_tensor(u[:], mu_r[:, c0:c0 + cn],
                                                   bias["sw1"][:], p[:],
                                                   ALU.mult, ALU.add)
                    ua = sbufs.tile([128, cn], dt.bfloat16, tag="ua")
                    nc.vector.tensor_tensor(ua[:], u[:], A_r[:, c0:c0 + cn], ALU.mult)
                    nc.scalar.activation(h1[:, c0:c0 + cn], ua[:], AF.Relu,
                                         bias=bias["b1"][:], scale=1.0)

                # --- W2 + residual -> y ---
                y = acts.tile([128, DT, TT], dt.bfloat16, tag="y")
                for o in range(DT):
                    for (c0, cn) in CHUNKS:
                        p = psums.tile([128, cn], dt.float32, tag="pc")
                        nc.tensor.matmul(p[:], w["w2"][:, 128 * o:128 * o + 128],
                                         h1[:, c0:c0 + cn], start=True, stop=False)
                        nc.tensor.matmul(p[:], smalls["ident"][:],
                                         rc[:, o, c0:c0 + cn], start=False, stop=True)
                        nc.scalar.activation(y[:, o, c0:c0 + cn], p[:], AF.Identity,
                                             bias=bias["b2"][:, o:o + 1], scale=1.0)

                # --- ln_out -> next cat ---
                sqy = _squares(nc, acts, y)
                mu_y, A_y = _ln_stats(nc, pools, smalls, y, sqy, eps_tile)
                cat_next = acts.tile([128, DT, TT], dt.bfloat16, tag="cat1" if l % 2 == 0 else "cat0")
                for d in range(DT):
                    xc = sbufs.tile([128, TT], dt.bfloat16, tag="xc")
                    if d < 2:
                        nc.gpsimd.tensor_tensor(xc[:], y[:, d], mu_y[:], ALU.subtract)
                    else:
                        nc.vector.tensor_tensor(xc[:], y[:, d], mu_y[:], ALU.subtract)
                    if degen:
                        nc.vector.tensor_tensor(cat_next[:, d], xc[:], A_y[:], ALU.mult)
                    else:
                        zd = sbufs.tile([128, TT], dt.bfloat16, tag="zd")
                        nc.vector.tensor_tensor(zd[:], xc[:], A_y[:], ALU.mult)
                        nc.vector.tensor_scalar(cat_next[:, d], zd[:],
                                                bias["go"][:, d:d + 1],
                                                bias["bo2"][:, d:d + 1],
                                                ALU.mult, ALU.add)
                cat = cat_next

            # --- mean-pool utterance tokens (pos 1..4 of each 5-block) ---
            out_sb = sbufs.tile([128, DT], dt.float32, tag="outsb")
            for d in range(DT):
                view = cat[:, d, :].rearrange("p (s j) -> p s j", j=5)[:, :, 1:5]
                nc.vector.tensor_reduce(out_sb[:, d:d + 1], view,
                                        axis=mybir.AxisListType.XY, op=ALU.add)
            nc.vector.tensor_scalar_mul(out_sb[:], out_sb[:], 1.0 / U)
            nc.sync.dma_start(out_dram, out_sb[:])
    nc.compile()
    return nc


def _build(degen):
    nc = bacc.Bacc("TRN2", target_bir_lowering=False, debug=False, num_devices=NCORES)
    return _trace(nc, degen)


def _prep(ins):
    """Compile (cached) and build per-core input maps. Returns (nc, in_maps)."""
    shared, degen = _host_prep(ins)
    idx = _tok_index()
    x = ins["x"].astype(np.float32)          # [B, T, D]
    xp = x[:, idx, :]                        # [B, TT, D]
    zp = _ln_host(xp)                        # host-side layer-0 ln_in
    xT = np.ascontiguousarray(xp.transpose(0, 2, 1)).astype(bf16)  # [B, D, TT]
    zT = np.ascontiguousarray(zp.transpose(0, 2, 1)).astype(bf16)
    if degen not in _COMPILED:
        _COMPILED[degen] = _build(degen)
    nc = _COMPILED[degen]
    in_maps = []
    for b in range(NCORES):
        m = dict(shared)
        if degen:
            for l in range(L):
                m.pop(f"go{l}", None); m.pop(f"bo2{l}", None)
        m["xT"] = xT[b]
        m["zT"] = zT[b]
        in_maps.append(m)
    return nc, in_maps


def kernel(**inputs):
    ins = {k: np.asarray(v) for k, v in inputs.items()}
    nc, in_maps = _prep(ins)
    res = bass_utils.run_bass_kernel_spmd(nc, in_maps, core_ids=list(range(NCORES)))
    outs = []
    for b in range(NCORES):
        o = res.results[b]["out"]            # [128, DT]
        outs.append(o.T.reshape(D))          # d = dtile*128 + p
    return np.stack(outs).astype(np.float32)


# revision 10
# speedup vs baseline: 1.1534x; 1.1534x over previous
"""Trainium2 Bass kernel for nn_BasicNet4 (Emformer encoder, sparse attention).

Strategy (v2):
  - Data-parallel over batch B=8 across 8 NeuronCores (weights replicated).
  - Tokens reordered host-side into segment-interleaved order:
    seg i -> [rc_i, u_{4i}..u_{4i+3}] (5 tokens x 256 segs = 1280), so
    attention is block-diagonal with 5x5 blocks.
  - Attention via SEGMENT-ALIGNED windows of 25 segs = 125 tokens (10 full
    windows + one 30-token remainder). No halos/edges; the in-window mask is
    one rank-26 matmul accumulated into each score chunk. Since the pa
    partition index is the within-window key slot, the softmax denominator is
    a single 125-partition column-sum over the full token width.
  - Activations transposed in SBUF [d on partitions (4x128), tokens free].
  - LN affines folded into weights host-side. Layer-0 ln_in computed on the
    HOST. When the LN affines are trivial (they are for the reference
    inputs; runtime-checked), ln_out output is already normalized, so
    ln_in for layers 1..3 vanishes on device (z = cat).
  - ff_ln folded into the FFN-W1 matmul: h1 = relu((W1@rc - s_w1*mu)*rstd+b1)
    using a rank-1 correction, so zf is never materialized.
  - All reciprocals via DVE reciprocal_approx_fast (fp32), not the slow
    iterative InstReciprocal.
  - bf16 matmul operands / residual stream, fp32 PSUM accumulation.
"""

import sys

sys.path.insert(0, "/opt/trn_rl_repo")

import numpy as np
import ml_dtypes

import concourse.bass as bass
import concourse.mybir as mybir
import concourse.tile as tile
from concourse import bass_utils, bacc

bf16 = ml_dtypes.bfloat16
dt = mybir.dt
AF = mybir.ActivationFunctionType
ALU = mybir.AluOpType

# Model config (hardcoded from the problem spec)
D, H, FFN, L = 512, 4, 128, 4
SEG, RC = 4, 1
B, T = 8, 1025
U = T - RC            # 1024
NSEG = U // SEG       # 256
TT = NSEG * (SEG + RC)  # 1280 interleaved tokens
DT = D // 128         # 4 d tiles
DH = D // H           # 128 (= one partition tile per head)
NCORES = 8
CHUNKS = [(0, 512), (512, 512), (1024, 256)]  # psum-bank-sized fp32 chunks

# Attention windows: 25 segments = 125 tokens each; last window = 6 segs = 30.
WW = 125
NWIN = 11
WIN = [(WW * w, WW if w < NWIN - 1 else TT - WW * (NWIN - 1)) for w in range(NWIN)]
# score/AV psum groups: windows per group, and their column spans
GROUPS = [(0, 4), (4, 4), (8, 3)]   # (first window, n windows)
GSPAN = [(WIN[w0][0], sum(WIN[w0 + i][1] for i in range(nw))) for (w0, nw) in GROUPS]

CBF = np.float32(bf16(np.float32(30000.0)))  # mask constant (exp(-30000) == 0)

_COMPILED = {}


def _tok_index():
    # interleaved token t -> original frame index in x[:, :T]
    t = np.arange(TT)
    seg = t // 5
    pos = t % 5
    off = np.array([4, 0, 1, 2, 3])[pos]
    return 4 * seg + off  # in [0, 1024]


def _mask_consts():
    """Window mask factors: psum += Lm.T @ Rm gives, for k-slot t and global
    query column j (window w = j//125, pos = j-125w):
        -C + C*[t//5 == pos//5]
    so allowed (same in-window segment) pairs get 0 and the rest -C.
    k-slots beyond the remainder window's 30 valid tokens get plain -C."""
    NS = WW // 5  # 25 segments per full window
    Lm = np.zeros((1 + NS, WW), np.float32)
    Rm = np.zeros((1 + NS, TT), np.float32)
    Lm[0] = 1.0
    for s in range(NS):
        Lm[1 + s, 5 * s:5 * s + 5] = 1.0
    j = np.arange(TT)
    w = np.minimum(j // WW, NWIN - 1)
    pos = j - WW * w
    Rm[0] = -CBF
    for s in range(NS):
        Rm[1 + s] = CBF * (pos // 5 == s)
    return Lm.astype(bf16), Rm.astype(bf16)


def _ln_host(x):
    mu = x.mean(-1, keepdims=True)
    var = ((x - mu) ** 2).mean(-1, keepdims=True)
    return (x - mu) / np.sqrt(var + 1e-5)


def _host_prep(ins):
    """Fold LN affines into weights, transpose, cast. Returns shared input map
    and whether the degenerate-affine fast path applies."""
    f32 = np.float32
    m = {}
    scale = np.float32(DH) ** -0.5
    degen = all(
        np.all(f32(ins[g]) == 1.0) and np.all(f32(ins[b]) == 0.0)
        for g, b in [("ln_in_g", "ln_in_b"), ("ln_out_g", "ln_out_b")]
    )
    for l in range(L):
        g_i, b_i = f32(ins["ln_in_g"][l]), f32(ins["ln_in_b"][l])
        g_f, b_f = f32(ins["ff_ln_g"][l]), f32(ins["ff_ln_b"][l])
        Wq = f32(ins["Wq"][l]);  bq = f32(ins["bq"][l])
        Wk = f32(ins["Wkv"][l][:D]);  bk = f32(ins["bkv"][l][:D])
        Wv = f32(ins["Wkv"][l][D:]);  bv = f32(ins["bkv"][l][D:])
        Wo = f32(ins["Wo"][l]);  bo = f32(ins["bo"][l])
        W1 = f32(ins["W1"][l]);  b1 = f32(ins["b1"][l])
        W2 = f32(ins["W2"][l]);  b2 = f32(ins["b2"][l])
        Wq_ = scale * (Wq * g_i[None, :]); bq_ = scale * (bq + Wq @ b_i)
        Wk_ = Wk * g_i[None, :];           bk_ = bk + Wk @ b_i
        Wv_ = Wv * g_i[None, :];           bv_ = bv + Wv @ b_i
        W1_ = W1 * g_f[None, :];           b1_ = b1 + W1 @ b_f
        m[f"wq{l}"] = Wq_.T.copy().astype(bf16)   # [din, dout]
        m[f"wk{l}"] = Wk_.T.copy().astype(bf16)
        m[f"wv{l}"] = Wv_.T.copy().astype(bf16)
        m[f"wo{l}"] = Wo.T.copy().astype(bf16)
        m[f"w1{l}"] = W1_.T.copy().astype(bf16)   # [512, 128]
        m[f"w2{l}"] = W2.T.copy().astype(bf16)    # [128, 512]
        m[f"bq{l}"] = bq_.reshape(DT, 128).T.copy()       # [128, DT] f32
        m[f"bk{l}"] = bk_.reshape(DT, 128).T.copy()
        # softmax rows sum to 1, so the V bias adds Wo@bv_ to every token:
        # fold it into bo and skip the V-bias matmul entirely.
        bo_ = bo + Wo @ bv_
        m[f"bo{l}"] = bo_.reshape(DT, 128).T.copy()
        m[f"b1{l}"] = b1_.reshape(1, 128).T.copy()        # [128, 1]
        m[f"sw1{l}"] = (-W1_.sum(axis=1)).reshape(1, 128).T.copy()  # [128, 1]
        m[f"b2{l}"] = b2.reshape(DT, 128).T.copy()
        m[f"go{l}"] = f32(ins["ln_out_g"][l]).reshape(DT, 128).T.copy()
        m[f"bo2{l}"] = f32(ins["ln_out_b"][l]).reshape(DT, 128).T.copy()
    Lm, Rm = _mask_consts()
    m["lm"] = Lm                                     # [26, 125]
    m["rm"] = Rm                                     # [26, 1280]
    m["ones_c"] = np.full((128, 128), 1.0 / D, bf16)  # stats lhsT (bcast)
    m["ones_w"] = np.ones((WW, 128), bf16)            # denominator lhsT
    m["ident"] = np.eye(128, dtype=bf16)              # residual adds
    return m, degen


def _dram_inputs(nc, degen):
    a = {}
    def inp(name, shape, dtype):
        a[name] = nc.dram_tensor(name, list(shape), dtype, kind="ExternalInput").ap()
    inp("xT", (D, TT), dt.bfloat16)
    inp("zT", (D, TT), dt.bfloat16)  # host-side ln_in of layer 0
    for l in range(L):
        inp(f"wq{l}", (D, D), dt.bfloat16); inp(f"wk{l}", (D, D), dt.bfloat16)
        inp(f"wv{l}", (D, D), dt.bfloat16); inp(f"wo{l}", (D, D), dt.bfloat16)
        inp(f"w1{l}", (D, FFN), dt.bfloat16); inp(f"w2{l}", (FFN, D), dt.bfloat16)
        inp(f"bq{l}", (128, DT), dt.float32); inp(f"bk{l}", (128, DT), dt.float32)
        inp(f"bo{l}", (128, DT), dt.float32)
        inp(f"b1{l}", (128, 1), dt.float32); inp(f"sw1{l}", (128, 1), dt.float32)
        inp(f"b2{l}", (128, DT), dt.float32)
        if not degen:
            inp(f"go{l}", (128, DT), dt.float32); inp(f"bo2{l}", (128, DT), dt.float32)
    inp("lm", (1 + WW // 5, WW), dt.bfloat16)
    inp("rm", (1 + WW // 5, TT), dt.bfloat16)
    inp("ones_c", (128, 128), dt.bfloat16)
    inp("ones_w", (WW, 128), dt.bfloat16)
    inp("ident", (128, 128), dt.bfloat16)
    out = nc.dram_tensor("out", [128, DT], dt.float32, kind="ExternalOutput").ap()
    return a, out


def _ln_stats(nc, pools, smalls, src, sq, eps_tile):
    """Given src [128, DT, TT] bf16 and its squares sq (same shape), return
    (mu_b bf16 [128, TT] broadcast, A fp32 [128, TT] broadcast = 1/std)."""
    acts, sbufs, psums = pools
    ones_c = smalls["ones_c"]
    mu_b = sbufs.tile([128, TT], dt.bfloat16, tag="mu")
    sqmu = sbufs.tile([128, TT], dt.float32, tag="sqmu")
    var = sbufs.tile([128, TT], dt.float32, tag="var")
    A = sbufs.tile([128, TT], dt.float32, tag="A")
    for (c0, cn) in CHUNKS:
        p_mu = psums.tile([128, cn], dt.float32, tag="pc")
        p_e2 = psums.tile([128, cn], dt.float32, tag="pc")
        for d in range(DT):
            nc.tensor.matmul(p_mu[:], ones_c[:], src[:, d, c0:c0 + cn],
                             start=(d == 0), stop=(d == DT - 1))
        for d in range(DT):
            nc.tensor.matmul(p_e2[:], ones_c[:], sq[:, d, c0:c0 + cn],
                             start=(d == 0), stop=(d == DT - 1))
        nc.vector.tensor_copy(mu_b[:, c0:c0 + cn], p_mu[:])
        nc.scalar.activation(sqmu[:, c0:c0 + cn], p_mu[:], AF.Square)
        nc.vector.tensor_tensor(var[:, c0:c0 + cn], p_e2[:], sqmu[:, c0:c0 + cn],
                                ALU.subtract)
        # std = sqrt(var + eps); A = 1/std
        nc.scalar.activation(sqmu[:, c0:c0 + cn], var[:, c0:c0 + cn], AF.Sqrt,
                             bias=eps_tile[:], scale=1.0)
        nc.vector.reciprocal_approx_fast(A[:, c0:c0 + cn], sqmu[:, c0:c0 + cn])
    return mu_b, A


def _squares(nc, acts, src, split=2):
    """sq = src*src elementwise, [128, DT, TT] bf16; split across engines,
    chunked so per-chunk stats can start before the full width is done."""
    sq = acts.tile([128, DT, TT], dt.bfloat16, tag="sq")
    for (c0, cn) in CHUNKS:
        for d in range(DT):
            s, v = src[:, d, c0:c0 + cn], sq[:, d, c0:c0 + cn]
            if d < split:
                nc.gpsimd.tensor_tensor(v, s, s, ALU.mult)
            else:
                nc.scalar.activation(v, s, AF.Square)
    return sq


def _trace(nc, degen):
    a, out_dram = _dram_inputs(nc, degen)
    with tile.TileContext(nc) as tc:
        import contextlib
        ctx = contextlib.ExitStack()
        with ctx:
            consts = ctx.enter_context(tc.tile_pool(name="consts", bufs=1))
            wpool = ctx.enter_context(tc.tile_pool(name="w", bufs=2))
            acts = ctx.enter_context(tc.tile_pool(name="acts", bufs=1))
            sbufs = ctx.enter_context(tc.tile_pool(name="sbufs", bufs=2))
            # PSUM budget: 8 banks of 2KB. pc ring x4 + pv x1 + ps x3.
            psums = ctx.enter_context(tc.tile_pool(name="psums", bufs=4, space="PSUM"))
            pvp = ctx.enter_context(tc.tile_pool(name="pvp", bufs=1, space="PSUM"))
            psp = ctx.enter_context(tc.tile_pool(name="psp", bufs=3, space="PSUM"))

            # constants
            smalls = {}
            for name, shape, dd in [
                ("lm", [1 + WW // 5, WW], dt.bfloat16),
                ("rm", [1 + WW // 5, TT], dt.bfloat16),
                ("ones_c", [128, 128], dt.bfloat16),
                ("ones_w", [WW, 128], dt.bfloat16),
                ("ident", [128, 128], dt.bfloat16),
            ]:
                t = consts.tile(shape, dd, tag=name)
                nc.sync.dma_start(t[:], a[name])
                smalls[name] = t
            eps_tile = consts.tile([128, 1], dt.float32)
            nc.vector.memset(eps_tile[:], 1e-5)

            # initial residual stream + host-normalized layer-0 LN input
            cat = acts.tile([128, DT, TT], dt.bfloat16, tag="cat0")
            nc.sync.dma_start(cat[:], a["xT"].rearrange("(dtile p) t -> p dtile t", p=128))
            z0 = acts.tile([128, DT, TT], dt.bfloat16, tag="z0")
            nc.sync.dma_start(z0[:], a["zT"].rearrange("(dtile p) t -> p dtile t", p=128))

            pools = (acts, sbufs, psums)

            # v_nat allocated once; zero it so the remainder-window pad rows
            # (never written by the V projection) give 0*0=0 in the AV matmul.
            v_nat = acts.tile([WW, NWIN, D], dt.bfloat16, tag="v")
            nc.gpsimd.memset(v_nat[:], 0.0)

            for l in range(L):
                # --- load layer weights ---
                w = {}
                for nm, shape in [("wq", [128, DT, D]), ("wk", [128, DT, D]),
                                  ("wv", [128, DT, D]), ("wo", [128, DT, D]),
                                  ("w1", [128, DT, FFN]), ("w2", [128, D])]:
                    t = wpool.tile(shape, dt.bfloat16, tag=nm)
                    src = a[f"{nm}{l}"]
                    if nm == "w2":
                        nc.sync.dma_start(t[:], src)
                    else:
                        nc.sync.dma_start(t[:], src.rearrange("(dtile p) o -> p dtile o", p=128))
                    w[nm] = t
                bias = {}
                bnames = ["bq", "bk", "bo", "b1", "sw1", "b2"]
                if not degen:
                    bnames += ["go", "bo2"]
                for nm in bnames:
                    t = wpool.tile([128, DT] if nm not in ("b1", "sw1") else [128, 1],
                                   dt.float32, tag=nm)
                    nc.sync.dma_start(t[:], a[f"{nm}{l}"])
                    bias[nm] = t

                # --- ln_in -> z ---
                if l == 0:
                    z = z0
                elif degen:
                    z = cat           # ln_out output is already normalized
                else:
                    sqc = _squares(nc, acts, cat)
                    mu_b, A = _ln_stats(nc, pools, smalls, cat, sqc, eps_tile)
                    z = acts.tile([128, DT, TT], dt.bfloat16, tag="z")
                    for d in range(DT):
                        xc = sbufs.tile([128, TT], dt.bfloat16, tag="xc")
                        nc.vector.tensor_tensor(xc[:], cat[:, d], mu_b[:], ALU.subtract)
                        nc.vector.tensor_tensor(z[:, d], xc[:], A[:], ALU.mult)

                # --- Q, K projections (weights stationary -> transposed out) ---
                qk = {}
                for nm, bnm in [("wq", "bq"), ("wk", "bk")]:
                    dst = acts.tile([128, DT, TT], dt.bfloat16,
                                    tag="q" if nm == "wq" else "k")
                    for o in range(DT):
                        for (c0, cn) in CHUNKS:
                            p = psums.tile([128, cn], dt.float32, tag="pc")
                            for d in range(DT):
                                nc.tensor.matmul(
                                    p[:],
                                    w[nm][:, d, 128 * o:128 * o + 128],
                                    z[:, d, c0:c0 + cn],
                                    start=(d == 0), stop=(d == DT - 1))
                            nc.scalar.activation(dst[:, o, c0:c0 + cn], p[:],
                                                 AF.Identity,
                                                 bias=bias[bnm][:, o:o + 1], scale=1.0)
                    qk[nm] = dst
                q_t, k_t = qk["wq"], qk["wk"]

                # --- V projection (acts stationary -> natural [tok, d]) ---
                for wi, (w0, wd) in enumerate(WIN):
                    p = pvp.tile([wd, D], dt.float32, tag="pv")
                    for d in range(DT):
                        nc.tensor.matmul(p[:], z[:, d, w0:w0 + wd],
                                         w["wv"][:, d, :], start=(d == 0),
                                         stop=(d == DT - 1))
                    nc.scalar.activation(v_nat[0:wd, wi, :], p[:], AF.Identity)

                # --- attention per head ---
                attn = acts.tile([128, DT, TT], dt.bfloat16, tag="attn")
                for h in range(H):
                    pa = sbufs.tile([WW, TT], dt.bfloat16, tag="pa")
                    # scores + mask, grouped windows per psum bank
                    for gi, (gw0, gnw) in enumerate(GROUPS):
                        g0, gn = GSPAN[gi]
                        ps = psp.tile([WW, gn], dt.float32, tag="ps")
                        nc.tensor.matmul(ps[:], smalls["lm"][:],
                                         smalls["rm"][:, g0:g0 + gn],
                                         start=True, stop=False)
                        for k in range(gnw):
                            w0, wd = WIN[gw0 + k]
                            lo = w0 - g0
                            nc.tensor.matmul(ps[0:wd, lo:lo + wd],
                                             k_t[:, h, w0:w0 + wd],
                                             q_t[:, h, w0:w0 + wd],
                                             start=False, stop=(k == gnw - 1),
                                             skip_group_check=True)
                        nc.scalar.activation(pa[:, g0:g0 + gn], ps[:], AF.Exp)
                    # denominator: plain partition sum over the window axis
                    rec = sbufs.tile([128, TT], dt.float32, tag="rec")
                    for (c0, cn) in CHUNKS:
                        pd = psums.tile([128, cn], dt.float32, tag="pc")
                        nc.tensor.matmul(pd[:], smalls["ones_w"][:],
                                         pa[:, c0:c0 + cn], start=True, stop=True)
                        nc.vector.reciprocal_approx_fast(rec[:, c0:c0 + cn], pd[:])
                    # attn-value matmuls (grouped like scores)
                    for gi, (gw0, gnw) in enumerate(GROUPS):
                        g0, gn = GSPAN[gi]
                        pv = psp.tile([128, gn], dt.float32, tag="ps")
                        for k in range(gnw):
                            w0, wd = WIN[gw0 + k]
                            lo = w0 - g0
                            nc.tensor.matmul(pv[:, lo:lo + wd],
                                             v_nat[:, gw0 + k, 128 * h:128 * h + 128],
                                             pa[:, w0:w0 + wd],
                                             start=True, stop=True)
                        nc.vector.tensor_tensor(attn[:, h, g0:g0 + gn], pv[:],
                                                rec[:, g0:g0 + gn], ALU.mult)

                # --- Wo projection + residual (psum + bo) + cat on gpsimd ---
                rc = acts.tile([128, DT, TT], dt.bfloat16, tag="rc")
                for o in range(DT):
                    for (c0, cn) in CHUNKS:
                        p = psums.tile([128, cn], dt.float32, tag="pc")
                        for d in range(DT):
                            nc.tensor.matmul(p[:],
                                             w["wo"][:, d, 128 * o:128 * o + 128],
                                             attn[:, d, c0:c0 + cn],
                                             start=(d == 0), stop=(d == DT - 1))
                        nc.vector.scalar_tensor_tensor(rc[:, o, c0:c0 + cn], p[:],
                                                       bias["bo"][:, o:o + 1],
                                                       cat[:, o, c0:c0 + cn],
                                                       ALU.add, ALU.add)

                # --- ff_ln folded into W1: h1 = relu((W1@rc - sw1*mu)*A + b1) ---
                sqr = _squares(nc, acts, rc)
                mu_r, A_r = _ln_stats(nc, pools, smalls, rc, sqr, eps_tile)
                h1 = acts.tile([128, TT], dt.bfloat16, tag="h1")
                for (c0, cn) in CHUNKS:
                    p = psums.tile([128, cn], dt.float32, tag="pc")
                    for d in range(DT):
                        nc.tensor.matmul(p[:], w["w1"][:, d, :],
                                         rc[:, d, c0:c0 + cn],
                                         start=(d == 0), stop=(d == DT - 1))
                    u = sbufs.tile([128, cn], dt.float32, tag="u")
                    nc.vector.scalar_tensor_tensor(u[:], mu_r[:, c0:c0 + cn],
                                                   bias["sw1"][:], p[:],
                                                   ALU.mult, ALU.add)
                    ua = sbufs.tile([128, cn], dt.bfloat16, tag="ua")
                    nc.vector.tensor_tensor(ua[:], u[:], A_r[:, c0:c0 + cn], ALU.mult)
                    nc.scalar.activation(h1[:, c0:c0 + cn], ua[:], AF.Relu,
                                         bias=bias["b1"][:], scale=1.0)

                # --- W2 + residual -> y ---
                y = acts.tile([128, DT, TT], dt.bfloat16, tag="y")
                for o in range(DT):
                    for (c0, cn) in CHUNKS:
                        p = psums.tile([128, cn], dt.float32, tag="pc")
                        nc.tensor.matmul(p[:], w["w2"][:, 128 * o:128 * o + 128],
                                         h1[:, c0:c0 + cn], start=True, stop=True)
                        nc.vector.scalar_tensor_tensor(y[:, o, c0:c0 + cn], p[:],
                                                       bias["b2"][:, o:o + 1],
                                                       rc[:, o, c0:c0 + cn],
                                                       ALU.add, ALU.add)

                # --- ln_out -> next cat ---
                sqy = _squares(nc, acts, y)
                mu_y, A_y = _ln_stats(nc, pools, smalls, y, sqy, eps_tile)
                cat_next = acts.tile([128, DT, TT], dt.bfloat16, tag="cat1" if l % 2 == 0 else "cat0")
                for (c0, cn) in CHUNKS:
                    for d in range(DT):
                        xc = sbufs.tile([128, cn], dt.bfloat16, tag="xc")
                        nc.vector.tensor_tensor(xc[:], y[:, d, c0:c0 + cn],
                                                mu_y[:, c0:c0 + cn], ALU.subtract)
                        if degen:
                            nc.vector.tensor_tensor(cat_next[:, d, c0:c0 + cn], xc[:],
                                                    A_y[:, c0:c0 + cn], ALU.mult)
                        else:
                            zd = sbufs.tile([128, cn], dt.bfloat16, tag="zd")
                            nc.vector.tensor_tensor(zd[:], xc[:],
                                                    A_y[:, c0:c0 + cn], ALU.mult)
                            nc.vector.tensor_scalar(cat_next[:, d, c0:c0 + cn], zd[:],
                                                    bias["go"][:, d:d + 1],
                                                    bias["bo2"][:, d:d + 1],
                                                    ALU.mult, ALU.add)
                cat = cat_next

            # --- mean-pool utterance tokens (pos 1..4 of each 5-block) ---
            out_sb = sbufs.tile([128, DT], dt.float32, tag="outsb")
            for d in range(DT):
                view = cat[:, d, :].rearrange("p (s j) -> p s j", j=5)[:, :, 1:5]
                nc.vector.tensor_reduce(out_sb[:, d:d + 1], view,
                                        axis=mybir.AxisListType.XY, op=ALU.add)
            nc.vector.tensor_scalar_mul(out_sb[:], out_sb[:], 1.0 / U)
            nc.sync.dma_start(out_dram, out_sb[:])
    nc.compile()
    return nc


def _build(degen):
    nc = bacc.Bacc("TRN2", target_bir_lowering=False, debug=False, num_devices=NCORES)
    return _trace(nc, degen)


def _prep(ins):
    """Compile (cached) and build per-core input maps. Returns (nc, in_maps)."""
    shared, degen = _host_prep(ins)
    idx = _tok_index()
    x = ins["x"].astype(np.float32)          # [B, T, D]
    xp = x[:, idx, :]                        # [B, TT, D]
    zp = _ln_host(xp)                        # host-side layer-0 ln_in
    xT = np.ascontiguousarray(xp.transpose(0, 2, 1)).astype(bf16)  # [B, D, TT]
    zT = np.ascontiguousarray(zp.transpose(0, 2, 1)).astype(bf16)
    if degen not in _COMPILED:
        _COMPILED[degen] = _build(degen)
    nc = _COMPILED[degen]
    in_maps = []
    for b in range(NCORES):
        m = dict(shared)
        if degen:
            for l in range(L):
                m.pop(f"go{l}", None); m.pop(f"bo2{l}", None)
        m["xT"] = xT[b]
        m["zT"] = zT[b]
        in_maps.append(m)
    return nc, in_maps


def kernel(**inputs):
    ins = {k: np.asarray(v) for k, v in inputs.items()}
    nc, in_maps = _prep(ins)
    res = bass_utils.run_bass_kernel_spmd(nc, in_maps, core_ids=list(range(NCORES)))
    outs = []
    for b in range(NCORES):
        o = res.results[b]["out"]            # [128, DT]
        outs.append(o.T.reshape(D))          # d = dtile*128 + p
    return np.stack(outs).astype(np.float32)
